# revision 45
# baseline (speedup 1.0000x reference)
"""MoE routing kernel for Trainium2 (8 NeuronCores, expert-parallel).

Problem: nn_MoDE_52140902973544 (moe_routing).
  x[4,2048,1024], router (8 experts, top-2, capacity 1024), 7 real experts
  with FFN H=1024 -> I=4096 -> H=1024 (relu), expert 7 = identity (noop).

Strategy:
  * Host: router forward + top-2 + capacity-limited dispatch (pure index
    math, order-based -> float-robust), gather dispatched tokens per
    expert transposed to [H, cap].
  * Device (SPMD over 8 cores): core e computes
        outT_e = (relu(disp_e @ Wi_e) @ Wo_e).T            # [H, cap]
    Core 7 duplicates core 0 (7 real experts); its output is ignored.
  * Host: combine via pure gathers (no scatter) + gate weights + noop path.

Device kernel layout (bf16 operands, fp32 PSUM):
  * x and h stay SBUF-resident; Wi arrives in 5 pieces (a tiny first piece
    so the PE starts after ~0.3MB of DMA) through a 4-slot ring that the
    4 Wo pieces rotate into as GEMM1 consumes them -> weight DMA fully
    overlaps compute.
  * GEMM1 interleaves the two 512-token PSUM tiles per i-chunk; GEMM2
    interleaves 4 PSUM tiles; outbound DMA is per-piece so only ~1MB of
    the fp32 output write is a serial tail.
  * Measured (slope method, see test.py): ~293us/execution at the chip's
    sustained-power clock. The same NEFF measured 209us/iter (0.96
    cycles/column at the full 2.4GHz) in a fresh-clock session right
    after a core reset — i.e. the kernel streams the PE at ~100%
    efficiency and the 293/209 gap is chip power-state throttle, not
    code. fp8 DoubleRow runs 2x faster but its e4m3 quantization noise
    (~4e-2 end-to-end) fails the 2e-2 gate, and residual-compensation
    schemes cost more than bf16.
"""

import os
import sys

for _p in ("/opt/trn_rl_repo", "/opt/pypackages"):
    if _p not in sys.path:
        sys.path.append(_p)

import numpy as np

# ---- problem constants (hardcoded per contract) ----
B, S, H, I = 4, 2048, 1024, 4096
E = 8                 # experts incl. noop (last)
ER = E - 1            # real experts
TOP_K = 2
N_TOK = B * S         # 8192
CAP = 1024            # ceil(N_TOK / E * 1.0)
N_CORES = 8

P = 128               # partitions
KO = H // P           # 8   H chunks
IC = I // P           # 32  I chunks
NF = 512              # matmul free dim
NN = CAP // NF        # 2   cap tiles

# matmul operand dtype: "bf16" (fast, host cast) or "fp8" (float8e4 +
# DoubleRow perf mode, ~2x PE throughput vs bf16 but ~4e-2 rel err —
# fails the 2e-2 gate; kept for experiments only)
MM_DTYPE = os.environ.get("MOE_MM_DTYPE", "bf16")
# fp8 pre-scales (powers of 2: exponent-only, no mantissa change) lift the
# small expert weights out of the fp8e4 subnormal range; the host combine
# divides the device output by WI_SCALE*WO_SCALE.
WI_SCALE = 16.0
WO_SCALE = 64.0

_CACHE = {}


def _build_nc(mm_dtype: str, repeat: int = 1, loop_repeat: int | None = None,
              staggered: bool = False, ablate: str = "full",
              x_one: bool = False, ps2bank: bool = True):
    """Build the single-core Bass program (SPMD across 8 cores).

    Layout: x [H,cap] and h [I,cap] stay SBUF-resident in bf16; Wi is
    loaded in 4 pieces which the Wo pieces rotate into (pool tag ring)
    as GEMM1 consumes them, so weight DMA fully overlaps compute and the
    SBUF footprint stays ~176 KB/partition.  GEMM2 accumulates the full
    I contraction in PSUM (4 banks live) and a single outbound DMA
    writes outT.
    """
    import concourse.bacc as bacc
    import concourse.mybir as mybir
    import concourse.tile as tile

    dt = mybir.dt
    assert mm_dtype in ("bf16", "fp8")
    fp8 = mm_dtype == "fp8"
    # fp8: operands in float8e4 (TRN e4m3, max +-240), matmuls in DoubleRow
    # perf mode (2 k-chunks per instruction, ~1.4x PE throughput). PSUM
    # accumulation stays fp32; host pre-scales wi by 16 and wo by 64 to
    # center the fp8 dynamic range, and the host combine divides the gate
    # weights by 1024 to compensate (relu commutes with positive scale).
    DT = dt.float8e4 if fp8 else dt.bfloat16
    KSTEP = 2 if fp8 else 1
    PERF = mybir.MatmulPerfMode.DoubleRow if fp8 else None

    # Bacc (not raw Bass): its compile() pipeline splits multi-semaphore
    # waits into event-semaphore chains (TRN2 allows 1 wait/instruction)
    # and moves matmul waits onto ldweights.
    nc = bacc.Bacc("TRN2")
    xT = nc.declare_dram_parameter("xT", [H, CAP], DT, isOutput=False)
    wi = nc.declare_dram_parameter("wi", [H, I], DT, isOutput=False)
    wo = nc.declare_dram_parameter("wo", [I, H], DT, isOutput=False)
    outT = nc.declare_dram_parameter("outT", [H, CAP], dt.float32, isOutput=True)
    NPIECE = 4
    IPP = IC // NPIECE        # 8 i-chunks per wi piece

    with tile.TileContext(nc) as tc:
        from contextlib import ExitStack

        with ExitStack() as ctx:
            xpool = ctx.enter_context(tc.tile_pool(name="x", bufs=1))
            wpool = ctx.enter_context(tc.tile_pool(name="w", bufs=NPIECE))
            hpool = ctx.enter_context(tc.tile_pool(name="h", bufs=1))
            opool = ctx.enter_context(tc.tile_pool(name="o", bufs=1))
            # one shared 8-bank PSUM ring: GEMM1 draws 4 banks per i-chunk
            # pair, GEMM2 draws 4 per group — consecutive groups land on
            # different bank halves, so the DVE copies of group g overlap
            # the PE matmuls of group g+1 instead of serializing on the
            # same banks. ps2bank: tiles span 2 banks (both n-halves), so
            # each relu/copy drains a full [128,1024] row in one DVE op.
            pspool = ctx.enter_context(
                tc.tile_pool(name="ps", bufs=(4 if ps2bank else 8),
                             space="PSUM"))

            # x split into the two 512-token halves: GEMM1's first chain
            # only waits on the first 1MB DMA, not the full 2MB
            # (x_one=True: single tile, A/B experiment for SBUF access
            # patterns — rhs then comes from one tile for all matmuls)
            if x_one:
                x_whole = xpool.tile([P, KO, CAP], DT, name="x")
                x_sbs = [x_whole[:, :, n * NF:(n + 1) * NF] for n in range(NN)]
            else:
                x_sbs = [xpool.tile([P, KO, NF], DT, name=f"x{n}")
                         for n in range(NN)]
            h_sb = hpool.tile([P, IC, CAP], DT)
            out_sb = opool.tile([P, KO, CAP], dt.float32)

            wi_r = wi.rearrange("(ko p) i -> p ko i", p=P)     # [128, 8, 4096]
            wo_r = wo.rearrange("(ki p) h -> p ki h", p=P)     # [128, 32, 1024]

          # fmt: off
          # noqa
            # wi piece sizes in i-chunks: a tiny first piece so the PE can
            # start GEMM1 after a ~0.5MB DMA instead of 4MB (the tile dep
            # is whole-tile); the ring still has NPIECE slots, with wi4
            # reusing wi0's slot after its chunks are consumed. All counts
            # even so GEMM1 can pair i-chunks for 4-bank interleaving.
            WI_SPLIT = [2, IC // NPIECE - 2] + [IC // NPIECE] * (NPIECE - 1)
            WI_START = [sum(WI_SPLIT[:j]) for j in range(len(WI_SPLIT))]

            xT_r = xT.rearrange("(ko p) n -> p ko n", p=P)

            # ablate: "full" | "empty" (loop overhead only) | "nog2"
            # (GEMM1+relu only) | "nog2pre" (GEMM1 only, wi preloaded
            # OUTSIDE the loop — separates DMA interference from relu
            # interference) | "notail" (skip PSUM->SBUF copies + out
            # DMA) | "nodma" (skip input DMAs; compute on stale SBUF) —
            # timing-attribution experiments, not for production use.
            do_dma = ablate != "nodma"
            do_g2 = ablate not in ("nog2", "nog2pre")
            do_tail = ablate not in ("nog2", "nog2pre", "notail")
            preload = ablate == "nog2pre"

            wi_pre = None
            if preload:
                # wi fully SBUF-resident (64KB/partition; fits since wo and
                # the w-ring go unused in this variant)
                wprepool = ctx.enter_context(
                    tc.tile_pool(name="wpre", bufs=1))
                wi_pre = []
                for p_, (i0, cnt) in enumerate(zip(WI_START, WI_SPLIT)):
                    wt = wprepool.tile([P, KO, cnt * P], DT,
                                       tag=f"wpre{p_}", name=f"wipre{p_}")
                    nc.sync.dma_start(
                        wt[:], wi_r[:, :, i0 * P:(i0 + cnt) * P])
                    wi_pre.append(wt)

            def _emit_body():
              if ablate == "empty":
                  nc.any.memset(out_sb[:, 0, 0:NF], 0.0)
                  return
              if do_dma:
                  if x_one:
                      nc.sync.dma_start(x_whole[:], xT_r[:])
                  else:
                      for n in range(NN):
                          nc.sync.dma_start(
                              x_sbs[n][:], xT_r[:, :, n * NF:(n + 1) * NF])
              if preload:
                  wi_pieces = wi_pre
              else:
                  wi_pieces = []
                  for p_, (i0, cnt) in enumerate(zip(WI_START, WI_SPLIT)):
                      wt = wpool.tile([P, KO, cnt * P], DT, tag="w",
                                      name=f"wi{p_}")
                      if do_dma:
                          nc.sync.dma_start(
                              wt[:], wi_r[:, :, i0 * P:(i0 + cnt) * P])
                      wi_pieces.append(wt)
              # ---- GEMM1: hT = relu(Wi.T @ X.T) ----
              wo_pieces = []
              HPP = H // NPIECE                                  # 256 H cols
              for p_, (i0, cnt) in enumerate(zip(WI_START, WI_SPLIT)):
                  wt = wi_pieces[p_]
                  for ir in range(0, cnt, 2):
                      # 4-way PSUM interleave over an i-chunk pair x the two
                      # n-tiles: consecutive matmuls hit 4 different banks
                      # (GEMM2's 4-deep pattern measures 238ns/matmul vs
                      # 305ns for the 2-deep version of this loop).
                      pair = [i0 + ir, i0 + ir + 1]
                      if ps2bank:
                          p2 = [pspool.tile([P, NN, NF], dt.float32,
                                            tag="ps", name=f"ps1_{i}")
                                for i in pair]
                          pts1 = [[p2[di][:, n, :] for n in range(NN)]
                                  for di in range(2)]
                      else:
                          pts1 = [
                              [
                                  pspool.tile([P, NF], dt.float32, tag="ps",
                                              name=f"ps1_{i}_{n}")
                                  for n in range(NN)
                              ]
                              for i in pair
                          ]
                      for k in range(0, KO, KSTEP):
                          for di in range(2):
                              for n in range(NN):
                                  nc.tensor.matmul(
                                      pts1[di][n][:],
                                      wt[:, k:k + KSTEP,
                                         (ir + di) * P:(ir + di + 1) * P],
                                      x_sbs[n][:, k:k + KSTEP, :],
                                      start=(k == 0),
                                      stop=(k == KO - KSTEP),
                                      perf_mode=PERF,
                                  )
                      for di, i in enumerate(pair):
                          if ps2bank:
                              nc.vector.tensor_scalar_max(
                                  h_sb[:, i, :], p2[di][:], 0.0)
                          else:
                              for n in range(NN):
                                  nc.vector.tensor_scalar_max(
                                      h_sb[:, i, n * NF:(n + 1) * NF],
                                      pts1[di][n][:], 0.0
                                  )
                  # piece p_ fully consumed -> rotate the next wo piece into
                  # the freed ring slot (5 wi pieces, 4 wo pieces: skip the
                  # tiny piece 0)
                  if p_ >= 1 and do_g2:
                      g = p_ - 1
                      wot = wpool.tile([P, IC, HPP], DT, tag="w",
                                       name=f"wo{g}")
                      if do_dma:
                          nc.sync.dma_start(
                              wot[:], wo_r[:, :, g * HPP:(g + 1) * HPP])
                      wo_pieces.append(wot)

              if not do_g2:
                  return
              # ---- GEMM2: outT = Wo.T @ hT ----
              HGM = HPP // P                                     # 2 m per piece
              for g in range(NPIECE):
                  wt = wo_pieces[g]
                  if ps2bank:
                      q2 = [pspool.tile([P, NN, NF], dt.float32, tag="ps",
                                        name=f"ps2_{g}_{m}")
                            for m in range(HGM)]
                      pts = [[q2[m][:, n, :] for n in range(NN)]
                             for m in range(HGM)]
                  else:
                      pts = [
                          [
                              pspool.tile([P, NF], dt.float32, tag="ps",
                                          name=f"ps2_{g}_{m}_{n}")
                              for n in range(NN)
                          ]
                          for m in range(HGM)
                      ]
                  for k in range(0, IC, KSTEP):
                      for m in range(HGM):
                          for n in range(NN):
                              nc.tensor.matmul(
                                  pts[m][n][:],
                                  wt[:, k:k + KSTEP, m * P:(m + 1) * P],
                                  h_sb[:, k:k + KSTEP, n * NF:(n + 1) * NF],
                                  start=(k == 0),
                                  stop=(k == IC - KSTEP),
                                  perf_mode=PERF,
                              )
                  if not do_tail:
                      continue
                  for m in range(HGM):
                      if ps2bank:
                          nc.vector.tensor_copy(
                              out_sb[:, g * HGM + m, :], q2[m][:])
                      else:
                          for n in range(NN):
                              nc.vector.tensor_copy(
                                  out_sb[:, g * HGM + m, n * NF:(n + 1) * NF],
                                  pts[m][n][:])
                  # per-piece outbound DMA: overlaps the remaining GEMM2
                  # pieces, so the serial tail is ~1MB instead of 4MB (the
                  # WAW chain across the 4 writes resolves in piece order)
                  nc.sync.dma_start(
                      outT.rearrange("(ko p) n -> p ko n", p=P)[
                          :, g * HGM:(g + 1) * HGM, :],
                      out_sb[:, g * HGM:(g + 1) * HGM, :])

            if loop_repeat is not None:
                # device-side repeat loop: used only for timing (the slope
                # d(wall)/d(R) isolates per-iteration device time from the
                # ~70ms axon dispatch round-trip). hint_engines arms the
                # back-edge branch prefetch (the >256-instruction body
                # otherwise stalls ~4us on the IRAM fetch).
                with tc.For_i(0, loop_repeat, 1,
                              hint_engines=(mybir.EngineType.PE,
                                            mybir.EngineType.DVE),
                              staggered_reset=staggered):
                    _emit_body()
            else:
                for _rep in range(repeat):
                    _emit_body()
    nc.compile()
    return nc


def _get_nc(mm_dtype: str):
    if mm_dtype not in _CACHE:
        _CACHE[mm_dtype] = _build_nc(mm_dtype)
    return _CACHE[mm_dtype]


def _routing(x_flat: np.ndarray, router_w: np.ndarray, router_b: np.ndarray):
    """Replicate the reference router bit-for-bit where possible (jax CPU),
    returning top-2 values/indices [N_TOK, 2] (fp32/int)."""
    try:
        import jax
        import jax.numpy as jnp

        cpu = jax.devices("cpu")[0]
        with jax.default_device(cpu):
            xj = jnp.asarray(x_flat.reshape(B, S, H))
            logits = jnp.einsum("bsh,eh->bse", xj, jnp.asarray(router_w)) \
                + jnp.asarray(router_b)
            wflat = jax.nn.softmax(logits, axis=-1).reshape(N_TOK, E)
            topv, topi = jax.lax.top_k(wflat, TOP_K)
            return np.asarray(topv), np.asarray(topi)
    except Exception:
        # numpy fallback (float64 logits for a stable ordering)
        logits = x_flat.astype(np.float64) @ router_w.astype(np.float64).T \
            + router_b.astype(np.float64)
        m = logits.max(axis=1, keepdims=True)
        ex = np.exp(logits - m)
        wflat = (ex / ex.sum(axis=1, keepdims=True)).astype(np.float32)
        topi = np.argsort(-wflat, axis=1, kind="stable")[:, :TOP_K]
        topv = np.take_along_axis(wflat, topi, axis=1)
        return topv, topi


def _dispatch(x_flat, topv, topi):
    """Capacity-limited dispatch (exact reference order semantics).

    Returns (pos, disp_T): pos[t, e] = rank of t among selectors of e
    (token order); disp_T[e] = x of the first CAP selectors, transposed
    to [H, CAP]."""
    mask = np.zeros((N_TOK, E), dtype=bool)
    rows = np.arange(N_TOK)
    mask[rows[:, None], topi] = True
    expert_mask = mask[:, :ER]                       # [N, 7]
    pos = np.cumsum(expert_mask, axis=0, dtype=np.int32) - 1

    disp_T = np.zeros((ER, H, CAP), dtype=np.float32)
    for e in range(ER):
        idx_e = np.nonzero(expert_mask[:, e])[0][:CAP]
        disp_T[e, :, :len(idx_e)] = x_flat[idx_e].T
    return pos, disp_T


def _make_in_maps(disp_T, experts_inter, experts_out, mm_dtype=None):
    """Per-core device input maps + the output scale to undo fp8 pre-scaling."""
    import ml_dtypes

    mm_dtype = mm_dtype or MM_DTYPE
    if mm_dtype == "fp8":
        f8 = ml_dtypes.float8_e4m3
        cast_x = lambda a: np.ascontiguousarray(a.astype(f8))
        cast_wi = lambda a: np.ascontiguousarray((a * WI_SCALE).astype(f8))
        cast_wo = lambda a: np.ascontiguousarray((a * WO_SCALE).astype(f8))
        out_scale = 1.0 / (WI_SCALE * WO_SCALE)
    else:
        bf = lambda a: np.ascontiguousarray(a.astype(ml_dtypes.bfloat16))
        cast_x = cast_wi = cast_wo = bf
        out_scale = 1.0

    in_maps = []
    for c in range(N_CORES):
        e = c if c < ER else 0
        in_maps.append({
            "xT": cast_x(disp_T[e]),
            "wi": cast_wi(experts_inter[e]),
            "wo": cast_wo(experts_out[e]),
        })
    return in_maps, out_scale


def kernel(x, router_w, router_b, experts_inter, experts_out):
    from concourse.bass_utils import run_bass_kernel_spmd

    x = np.ascontiguousarray(np.asarray(x, dtype=np.float32))
    router_w = np.asarray(router_w, dtype=np.float32)
    router_b = np.asarray(router_b, dtype=np.float32)
    experts_inter = np.asarray(experts_inter, dtype=np.float32)
    experts_out = np.asarray(experts_out, dtype=np.float32)

    x_flat = x.reshape(N_TOK, H)
    topv, topi = _routing(x_flat, router_w, router_b)
    pos, disp_T = _dispatch(x_flat, topv, topi)
    rows = np.arange(N_TOK)

    mm_dtype = MM_DTYPE
    in_maps, out_scale = _make_in_maps(disp_T, experts_inter, experts_out,
                                       mm_dtype)

    nc = _get_nc(mm_dtype)
    trace = bool(int(os.environ.get("MOE_TRACE", "0")))
    res = run_bass_kernel_spmd(nc, in_maps, list(range(N_CORES)), trace=trace)
    global LAST_RESULT
    LAST_RESULT = res
    out_T = np.stack([res.results[e]["outT"] for e in range(ER)])  # [7,H,cap]

    # ---- host combine: pure gathers ----
    out_flat = np.ascontiguousarray(out_T.transpose(0, 2, 1)).reshape(
        ER * CAP, H)
    out_ext = np.vstack([out_flat, np.zeros((1, H), dtype=np.float32)])

    combined = np.zeros_like(x_flat)
    noop_w = np.zeros(N_TOK, dtype=np.float32)
    for k in range(TOP_K):
        e_k = topi[:, k]
        v_k = topv[:, k]
        is_noop = e_k == ER
        noop_w += np.where(is_noop, v_k, 0.0).astype(np.float32)
        p_k = pos[rows, np.minimum(e_k, ER - 1)]
        ok = (~is_noop) & (p_k < CAP)
        slot = np.where(ok, np.minimum(e_k, ER - 1) * CAP + p_k, ER * CAP)
        combined += out_ext[slot] * (np.where(ok, v_k, 0.0) * out_scale)[:, None]
    combined += x_flat * noop_w[:, None]

    return combined.reshape(B, S, H)



# revision 49
# speedup vs baseline: 1.0064x; 1.0064x over previous
"""MoE routing kernel for Trainium2 (8 NeuronCores, expert-parallel).

Problem: nn_MoDE_52140902973544 (moe_routing).
  x[4,2048,1024], router (8 experts, top-2, capacity 1024), 7 real experts
  with FFN H=1024 -> I=4096 -> H=1024 (relu), expert 7 = identity (noop).

Strategy:
  * Host: router forward + top-2 + capacity-limited dispatch (pure index
    math, order-based -> float-robust), gather dispatched tokens per
    expert transposed to [H, cap].
  * Device (SPMD over 8 cores): core e computes
        outT_e = (relu(disp_e @ Wi_e) @ Wo_e).T            # [H, cap]
    Core 7 duplicates core 0 (7 real experts); its output is ignored.
  * Host: combine via pure gathers (no scatter) + gate weights + noop path.

Device kernel layout (bf16 operands, fp32 PSUM):
  * x and h stay SBUF-resident; Wi arrives in 5 pieces (a tiny first piece
    so the PE starts after ~0.3MB of DMA) through a 4-slot ring that the
    4 Wo pieces rotate into as GEMM1 consumes them -> weight DMA fully
    overlaps compute.
  * GEMM1 interleaves the two 512-token PSUM tiles per i-chunk; GEMM2
    interleaves 4 PSUM tiles; outbound DMA is per-piece so only ~1MB of
    the fp32 output write is a serial tail.
  * Measured (slope method, see test.py): ~293us/execution at the chip's
    sustained-power clock. The same NEFF measured 209us/iter (0.96
    cycles/column at the full 2.4GHz) in a fresh-clock session right
    after a core reset — i.e. the kernel streams the PE at ~100%
    efficiency and the 293/209 gap is chip power-state throttle, not
    code. fp8 DoubleRow runs 2x faster but its e4m3 quantization noise
    (~4e-2 end-to-end) fails the 2e-2 gate, and residual-compensation
    schemes cost more than bf16.
"""

import os
import sys

for _p in ("/opt/trn_rl_repo", "/opt/pypackages"):
    if _p not in sys.path:
        sys.path.append(_p)

import numpy as np

# ---- problem constants (hardcoded per contract) ----
B, S, H, I = 4, 2048, 1024, 4096
E = 8                 # experts incl. noop (last)
ER = E - 1            # real experts
TOP_K = 2
N_TOK = B * S         # 8192
CAP = 1024            # ceil(N_TOK / E * 1.0)
N_CORES = 8

P = 128               # partitions
KO = H // P           # 8   H chunks
IC = I // P           # 32  I chunks
NF = 512              # matmul free dim
NN = CAP // NF        # 2   cap tiles

# matmul operand dtype: "bf16" (fast, host cast) or "fp8" (float8e4 +
# DoubleRow perf mode, ~2x PE throughput vs bf16 but ~4e-2 rel err —
# fails the 2e-2 gate; kept for experiments only)
MM_DTYPE = os.environ.get("MOE_MM_DTYPE", "bf16")
# fp8 pre-scales (powers of 2: exponent-only, no mantissa change) lift the
# small expert weights out of the fp8e4 subnormal range; the host combine
# divides the device output by WI_SCALE*WO_SCALE.
WI_SCALE = 16.0
WO_SCALE = 64.0

_CACHE = {}


def _build_nc(mm_dtype: str, repeat: int = 1, loop_repeat: int | None = None,
              staggered: bool = False, ablate: str = "full",
              x_one: bool = False, ps2bank: bool = True):
    """Build the single-core Bass program (SPMD across 8 cores).

    Layout: x [H,cap] and h [I,cap] stay SBUF-resident in bf16; Wi is
    loaded in 4 pieces which the Wo pieces rotate into (pool tag ring)
    as GEMM1 consumes them, so weight DMA fully overlaps compute and the
    SBUF footprint stays ~176 KB/partition.  GEMM2 accumulates the full
    I contraction in PSUM (4 banks live) and a single outbound DMA
    writes outT.
    """
    import concourse.bacc as bacc
    import concourse.mybir as mybir
    import concourse.tile as tile

    dt = mybir.dt
    assert mm_dtype in ("bf16", "fp8")
    fp8 = mm_dtype == "fp8"
    # fp8: operands in float8e4 (TRN e4m3, max +-240), matmuls in DoubleRow
    # perf mode (2 k-chunks per instruction, ~1.4x PE throughput). PSUM
    # accumulation stays fp32; host pre-scales wi by 16 and wo by 64 to
    # center the fp8 dynamic range, and the host combine divides the gate
    # weights by 1024 to compensate (relu commutes with positive scale).
    DT = dt.float8e4 if fp8 else dt.bfloat16
    KSTEP = 2 if fp8 else 1
    PERF = mybir.MatmulPerfMode.DoubleRow if fp8 else None

    # Bacc (not raw Bass): its compile() pipeline splits multi-semaphore
    # waits into event-semaphore chains (TRN2 allows 1 wait/instruction)
    # and moves matmul waits onto ldweights.
    nc = bacc.Bacc("TRN2")
    xT = nc.declare_dram_parameter("xT", [H, CAP], DT, isOutput=False)
    wi = nc.declare_dram_parameter("wi", [H, I], DT, isOutput=False)
    wo = nc.declare_dram_parameter("wo", [I, H], DT, isOutput=False)
    outT = nc.declare_dram_parameter("outT", [H, CAP], dt.float32, isOutput=True)
    NPIECE = 4
    IPP = IC // NPIECE        # 8 i-chunks per wi piece

    with tile.TileContext(nc) as tc:
        from contextlib import ExitStack

        with ExitStack() as ctx:
            xpool = ctx.enter_context(tc.tile_pool(name="x", bufs=1))
            wpool = ctx.enter_context(tc.tile_pool(name="w", bufs=NPIECE))
            hpool = ctx.enter_context(tc.tile_pool(name="h", bufs=1))
            opool = ctx.enter_context(tc.tile_pool(name="o", bufs=1))
            # one shared 8-bank PSUM ring: GEMM1 draws 4 banks per i-chunk
            # pair, GEMM2 draws 4 per group — consecutive groups land on
            # different bank halves, so the DVE copies of group g overlap
            # the PE matmuls of group g+1 instead of serializing on the
            # same banks. ps2bank: tiles span 2 banks (both n-halves), so
            # each relu/copy drains a full [128,1024] row in one DVE op.
            pspool = ctx.enter_context(
                tc.tile_pool(name="ps", bufs=(4 if ps2bank else 8),
                             space="PSUM"))

            # x split into the two 512-token halves: GEMM1's first chain
            # only waits on the first 1MB DMA, not the full 2MB
            # (x_one=True: single tile, A/B experiment for SBUF access
            # patterns — rhs then comes from one tile for all matmuls)
            if x_one:
                x_whole = xpool.tile([P, KO, CAP], DT, name="x")
                x_sbs = [x_whole[:, :, n * NF:(n + 1) * NF] for n in range(NN)]
            else:
                x_sbs = [xpool.tile([P, KO, NF], DT, name=f"x{n}")
                         for n in range(NN)]
            # h split into two half-tiles: GEMM2's k=0..15 matmuls wait only
            # on the first half's relus (done ~70us before GEMM1 ends), so
            # the GEMM1->GEMM2 transition has no whole-tile barrier
            ICH = IC // 2
            h_sbs = [hpool.tile([P, ICH, CAP], DT, name=f"h{j}")
                     for j in range(2)]
            out_sb = opool.tile([P, KO, CAP], dt.float32)

            wi_r = wi.rearrange("(ko p) i -> p ko i", p=P)     # [128, 8, 4096]
            wo_r = wo.rearrange("(ki p) h -> p ki h", p=P)     # [128, 32, 1024]

          # fmt: off
          # noqa
            # wi piece sizes in i-chunks: a tiny first piece so the PE can
            # start GEMM1 after a ~0.5MB DMA instead of 4MB (the tile dep
            # is whole-tile); the ring still has NPIECE slots, with wi4
            # reusing wi0's slot after its chunks are consumed. All counts
            # even so GEMM1 can pair i-chunks for 4-bank interleaving.
            WI_SPLIT = [2, IC // NPIECE - 2] + [IC // NPIECE] * (NPIECE - 1)
            WI_START = [sum(WI_SPLIT[:j]) for j in range(len(WI_SPLIT))]

            xT_r = xT.rearrange("(ko p) n -> p ko n", p=P)

            # ablate: "full" | "empty" (loop overhead only) | "nog2"
            # (GEMM1+relu only) | "nog2pre" (GEMM1 only, wi preloaded
            # OUTSIDE the loop — separates DMA interference from relu
            # interference) | "notail" (skip PSUM->SBUF copies + out
            # DMA) | "nodma" (skip input DMAs; compute on stale SBUF) —
            # timing-attribution experiments, not for production use.
            do_dma = ablate != "nodma"
            do_g2 = ablate not in ("nog2", "nog2pre")
            do_tail = ablate not in ("nog2", "nog2pre", "notail")
            preload = ablate == "nog2pre"

            wi_pre = None
            if preload:
                # wi fully SBUF-resident (64KB/partition; fits since wo and
                # the w-ring go unused in this variant)
                wprepool = ctx.enter_context(
                    tc.tile_pool(name="wpre", bufs=1))
                wi_pre = []
                for p_, (i0, cnt) in enumerate(zip(WI_START, WI_SPLIT)):
                    wt = wprepool.tile([P, KO, cnt * P], DT,
                                       tag=f"wpre{p_}", name=f"wipre{p_}")
                    nc.sync.dma_start(
                        wt[:], wi_r[:, :, i0 * P:(i0 + cnt) * P])
                    wi_pre.append(wt)

            def _emit_body():
              if ablate == "empty":
                  nc.any.memset(out_sb[:, 0, 0:NF], 0.0)
                  return
              if do_dma:
                  if x_one:
                      nc.sync.dma_start(x_whole[:], xT_r[:])
                  else:
                      for n in range(NN):
                          nc.sync.dma_start(
                              x_sbs[n][:], xT_r[:, :, n * NF:(n + 1) * NF])
              if preload:
                  wi_pieces = wi_pre
              else:
                  wi_pieces = []
                  for p_, (i0, cnt) in enumerate(zip(WI_START, WI_SPLIT)):
                      wt = wpool.tile([P, KO, cnt * P], DT, tag="w",
                                      name=f"wi{p_}")
                      if do_dma:
                          nc.sync.dma_start(
                              wt[:], wi_r[:, :, i0 * P:(i0 + cnt) * P])
                      wi_pieces.append(wt)
              # ---- GEMM1: hT = relu(Wi.T @ X.T) ----
              wo_pieces = []
              HPP = H // NPIECE                                  # 256 H cols
              for p_, (i0, cnt) in enumerate(zip(WI_START, WI_SPLIT)):
                  wt = wi_pieces[p_]
                  for ir in range(0, cnt, 2):
                      # 4-way PSUM interleave over an i-chunk pair x the two
                      # n-tiles: consecutive matmuls hit 4 different banks
                      # (GEMM2's 4-deep pattern measures 238ns/matmul vs
                      # 305ns for the 2-deep version of this loop).
                      pair = [i0 + ir, i0 + ir + 1]
                      if ps2bank:
                          p2 = [pspool.tile([P, NN, NF], dt.float32,
                                            tag="ps", name=f"ps1_{i}")
                                for i in pair]
                          pts1 = [[p2[di][:, n, :] for n in range(NN)]
                                  for di in range(2)]
                      else:
                          pts1 = [
                              [
                                  pspool.tile([P, NF], dt.float32, tag="ps",
                                              name=f"ps1_{i}_{n}")
                                  for n in range(NN)
                              ]
                              for i in pair
                          ]
                      for k in range(0, KO, KSTEP):
                          for di in range(2):
                              for n in range(NN):
                                  nc.tensor.matmul(
                                      pts1[di][n][:],
                                      wt[:, k:k + KSTEP,
                                         (ir + di) * P:(ir + di + 1) * P],
                                      x_sbs[n][:, k:k + KSTEP, :],
                                      start=(k == 0),
                                      stop=(k == KO - KSTEP),
                                      perf_mode=PERF,
                                  )
                      for di, i in enumerate(pair):
                          hdst = h_sbs[i // ICH]
                          if ps2bank:
                              nc.vector.tensor_scalar_max(
                                  hdst[:, i % ICH, :], p2[di][:], 0.0)
                          else:
                              for n in range(NN):
                                  nc.vector.tensor_scalar_max(
                                      hdst[:, i % ICH, n * NF:(n + 1) * NF],
                                      pts1[di][n][:], 0.0
                                  )
                  # piece p_ fully consumed -> rotate the next wo piece into
                  # the freed ring slot (5 wi pieces, 4 wo pieces: skip the
                  # tiny piece 0)
                  if p_ >= 1 and do_g2:
                      g = p_ - 1
                      wot = wpool.tile([P, IC, HPP], DT, tag="w",
                                       name=f"wo{g}")
                      if do_dma:
                          nc.sync.dma_start(
                              wot[:], wo_r[:, :, g * HPP:(g + 1) * HPP])
                      wo_pieces.append(wot)

              if not do_g2:
                  return
              # ---- GEMM2: outT = Wo.T @ hT ----
              HGM = HPP // P                                     # 2 m per piece
              for g in range(NPIECE):
                  wt = wo_pieces[g]
                  if ps2bank:
                      q2 = [pspool.tile([P, NN, NF], dt.float32, tag="ps",
                                        name=f"ps2_{g}_{m}")
                            for m in range(HGM)]
                      pts = [[q2[m][:, n, :] for n in range(NN)]
                             for m in range(HGM)]
                  else:
                      pts = [
                          [
                              pspool.tile([P, NF], dt.float32, tag="ps",
                                          name=f"ps2_{g}_{m}_{n}")
                              for n in range(NN)
                          ]
                          for m in range(HGM)
                      ]
                  for k in range(0, IC, KSTEP):
                      for m in range(HGM):
                          for n in range(NN):
                              nc.tensor.matmul(
                                  pts[m][n][:],
                                  wt[:, k:k + KSTEP, m * P:(m + 1) * P],
                                  h_sbs[k // ICH][
                                      :, k % ICH:k % ICH + KSTEP,
                                      n * NF:(n + 1) * NF],
                                  start=(k == 0),
                                  stop=(k == IC - KSTEP),
                                  perf_mode=PERF,
                              )
                  if not do_tail:
                      continue
                  # per-m copy + outbound DMA: each 0.5MB row leaves as soon
                  # as its copy lands, so only the last row's copy+DMA is a
                  # serial tail (~1.8us instead of ~3.5)
                  for m in range(HGM):
                      if ps2bank:
                          nc.vector.tensor_copy(
                              out_sb[:, g * HGM + m, :], q2[m][:])
                      else:
                          for n in range(NN):
                              nc.vector.tensor_copy(
                                  out_sb[:, g * HGM + m, n * NF:(n + 1) * NF],
                                  pts[m][n][:])
                      nc.sync.dma_start(
                          outT.rearrange("(ko p) n -> p ko n", p=P)[
                              :, g * HGM + m:g * HGM + m + 1, :],
                          out_sb[:, g * HGM + m:g * HGM + m + 1, :])

            if loop_repeat is not None:
                # device-side repeat loop: used only for timing (the slope
                # d(wall)/d(R) isolates per-iteration device time from the
                # ~70ms axon dispatch round-trip). hint_engines arms the
                # back-edge branch prefetch (the >256-instruction body
                # otherwise stalls ~4us on the IRAM fetch).
                with tc.For_i(0, loop_repeat, 1,
                              hint_engines=(mybir.EngineType.PE,
                                            mybir.EngineType.DVE),
                              staggered_reset=staggered):
                    _emit_body()
            else:
                for _rep in range(repeat):
                    _emit_body()
    nc.compile()
    return nc


def _get_nc(mm_dtype: str):
    if mm_dtype not in _CACHE:
        _CACHE[mm_dtype] = _build_nc(mm_dtype)
    return _CACHE[mm_dtype]


def _routing(x_flat: np.ndarray, router_w: np.ndarray, router_b: np.ndarray):
    """Replicate the reference router bit-for-bit where possible (jax CPU),
    returning top-2 values/indices [N_TOK, 2] (fp32/int)."""
    try:
        import jax
        import jax.numpy as jnp

        cpu = jax.devices("cpu")[0]
        with jax.default_device(cpu):
            xj = jnp.asarray(x_flat.reshape(B, S, H))
            logits = jnp.einsum("bsh,eh->bse", xj, jnp.asarray(router_w)) \
                + jnp.asarray(router_b)
            wflat = jax.nn.softmax(logits, axis=-1).reshape(N_TOK, E)
            topv, topi = jax.lax.top_k(wflat, TOP_K)
            return np.asarray(topv), np.asarray(topi)
    except Exception:
        # numpy fallback (float64 logits for a stable ordering)
        logits = x_flat.astype(np.float64) @ router_w.astype(np.float64).T \
            + router_b.astype(np.float64)
        m = logits.max(axis=1, keepdims=True)
        ex = np.exp(logits - m)
        wflat = (ex / ex.sum(axis=1, keepdims=True)).astype(np.float32)
        topi = np.argsort(-wflat, axis=1, kind="stable")[:, :TOP_K]
        topv = np.take_along_axis(wflat, topi, axis=1)
        return topv, topi


def _dispatch(x_flat, topv, topi):
    """Capacity-limited dispatch (exact reference order semantics).

    Returns (pos, disp_T): pos[t, e] = rank of t among selectors of e
    (token order); disp_T[e] = x of the first CAP selectors, transposed
    to [H, CAP]."""
    mask = np.zeros((N_TOK, E), dtype=bool)
    rows = np.arange(N_TOK)
    mask[rows[:, None], topi] = True
    expert_mask = mask[:, :ER]                       # [N, 7]
    pos = np.cumsum(expert_mask, axis=0, dtype=np.int32) - 1

    disp_T = np.zeros((ER, H, CAP), dtype=np.float32)
    for e in range(ER):
        idx_e = np.nonzero(expert_mask[:, e])[0][:CAP]
        disp_T[e, :, :len(idx_e)] = x_flat[idx_e].T
    return pos, disp_T


def _make_in_maps(disp_T, experts_inter, experts_out, mm_dtype=None):
    """Per-core device input maps + the output scale to undo fp8 pre-scaling."""
    import ml_dtypes

    mm_dtype = mm_dtype or MM_DTYPE
    if mm_dtype == "fp8":
        f8 = ml_dtypes.float8_e4m3
        cast_x = lambda a: np.ascontiguousarray(a.astype(f8))
        cast_wi = lambda a: np.ascontiguousarray((a * WI_SCALE).astype(f8))
        cast_wo = lambda a: np.ascontiguousarray((a * WO_SCALE).astype(f8))
        out_scale = 1.0 / (WI_SCALE * WO_SCALE)
    else:
        bf = lambda a: np.ascontiguousarray(a.astype(ml_dtypes.bfloat16))
        cast_x = cast_wi = cast_wo = bf
        out_scale = 1.0

    in_maps = []
    for c in range(N_CORES):
        e = c if c < ER else 0
        in_maps.append({
            "xT": cast_x(disp_T[e]),
            "wi": cast_wi(experts_inter[e]),
            "wo": cast_wo(experts_out[e]),
        })
    return in_maps, out_scale


def kernel(x, router_w, router_b, experts_inter, experts_out):
    from concourse.bass_utils import run_bass_kernel_spmd

    x = np.ascontiguousarray(np.asarray(x, dtype=np.float32))
    router_w = np.asarray(router_w, dtype=np.float32)
    router_b = np.asarray(router_b, dtype=np.float32)
    experts_inter = np.asarray(experts_inter, dtype=np.float32)
    experts_out = np.asarray(experts_out, dtype=np.float32)

    x_flat = x.reshape(N_TOK, H)
    topv, topi = _routing(x_flat, router_w, router_b)
    pos, disp_T = _dispatch(x_flat, topv, topi)
    rows = np.arange(N_TOK)

    mm_dtype = MM_DTYPE
    in_maps, out_scale = _make_in_maps(disp_T, experts_inter, experts_out,
                                       mm_dtype)

    nc = _get_nc(mm_dtype)
    trace = bool(int(os.environ.get("MOE_TRACE", "0")))
    res = run_bass_kernel_spmd(nc, in_maps, list(range(N_CORES)), trace=trace)
    global LAST_RESULT
    LAST_RESULT = res
    out_T = np.stack([res.results[e]["outT"] for e in range(ER)])  # [7,H,cap]

    # ---- host combine: pure gathers ----
    out_flat = np.ascontiguousarray(out_T.transpose(0, 2, 1)).reshape(
        ER * CAP, H)
    out_ext = np.vstack([out_flat, np.zeros((1, H), dtype=np.float32)])

    combined = np.zeros_like(x_flat)
    noop_w = np.zeros(N_TOK, dtype=np.float32)
    for k in range(TOP_K):
        e_k = topi[:, k]
        v_k = topv[:, k]
        is_noop = e_k == ER
        noop_w += np.where(is_noop, v_k, 0.0).astype(np.float32)
        p_k = pos[rows, np.minimum(e_k, ER - 1)]
        ok = (~is_noop) & (p_k < CAP)
        slot = np.where(ok, np.minimum(e_k, ER - 1) * CAP + p_k, ER * CAP)
        combined += out_ext[slot] * (np.where(ok, v_k, 0.0) * out_scale)[:, None]
    combined += x_flat * noop_w[:, None]

    return combined.reshape(B, S, H)



# revision 52
# speedup vs baseline: 1.0090x; 1.0026x over previous
"""MoE routing kernel for Trainium2 (8 NeuronCores, expert-parallel).

Problem: nn_MoDE_52140902973544 (moe_routing).
  x[4,2048,1024], router (8 experts, top-2, capacity 1024), 7 real experts
  with FFN H=1024 -> I=4096 -> H=1024 (relu), expert 7 = identity (noop).

Strategy:
  * Host: router forward + top-2 + capacity-limited dispatch (pure index
    math, order-based -> float-robust), gather dispatched tokens per
    expert transposed to [H, cap].
  * Device (SPMD over 8 cores): core e computes
        outT_e = (relu(disp_e @ Wi_e) @ Wo_e).T            # [H, cap]
    Core 7 duplicates core 0 (7 real experts); its output is ignored.
  * Host: combine via pure gathers (no scatter) + gate weights + noop path.

Device kernel layout (bf16 operands, fp32 PSUM):
  * x and h stay SBUF-resident; Wi arrives in 5 pieces (a tiny first piece
    so the PE starts after ~0.3MB of DMA) through a 4-slot ring that the
    4 Wo pieces rotate into as GEMM1 consumes them -> weight DMA fully
    overlaps compute.
  * GEMM1 interleaves the two 512-token PSUM tiles per i-chunk; GEMM2
    interleaves 4 PSUM tiles; outbound DMA is per-piece so only ~1MB of
    the fp32 output write is a serial tail.
  * Measured (slope method, see test.py): ~293us/execution at the chip's
    sustained-power clock. The same NEFF measured 209us/iter (0.96
    cycles/column at the full 2.4GHz) in a fresh-clock session right
    after a core reset — i.e. the kernel streams the PE at ~100%
    efficiency and the 293/209 gap is chip power-state throttle, not
    code. fp8 DoubleRow runs 2x faster but its e4m3 quantization noise
    (~4e-2 end-to-end) fails the 2e-2 gate, and residual-compensation
    schemes cost more than bf16.
"""

import os
import sys

for _p in ("/opt/trn_rl_repo", "/opt/pypackages"):
    if _p not in sys.path:
        sys.path.append(_p)

import numpy as np

# ---- problem constants (hardcoded per contract) ----
B, S, H, I = 4, 2048, 1024, 4096
E = 8                 # experts incl. noop (last)
ER = E - 1            # real experts
TOP_K = 2
N_TOK = B * S         # 8192
CAP = 1024            # ceil(N_TOK / E * 1.0)
N_CORES = 8

P = 128               # partitions
KO = H // P           # 8   H chunks
IC = I // P           # 32  I chunks
NF = 512              # matmul free dim
NN = CAP // NF        # 2   cap tiles

# matmul operand dtype: "bf16" (fast, host cast) or "fp8" (float8e4 +
# DoubleRow perf mode, ~2x PE throughput vs bf16 but ~4e-2 rel err —
# fails the 2e-2 gate; kept for experiments only)
MM_DTYPE = os.environ.get("MOE_MM_DTYPE", "bf16")
# fp8 pre-scales (powers of 2: exponent-only, no mantissa change) lift the
# small expert weights out of the fp8e4 subnormal range; the host combine
# divides the device output by WI_SCALE*WO_SCALE.
WI_SCALE = 16.0
WO_SCALE = 64.0

_CACHE = {}


def _build_nc(mm_dtype: str, repeat: int = 1, loop_repeat: int | None = None,
              staggered: bool = False, ablate: str = "full",
              x_one: bool = False, ps2bank: bool = True,
              wide_mm: bool = False):
    """Build the single-core Bass program (SPMD across 8 cores).

    Layout: x [H,cap] and h [I,cap] stay SBUF-resident in bf16; Wi is
    loaded in 4 pieces which the Wo pieces rotate into (pool tag ring)
    as GEMM1 consumes them, so weight DMA fully overlaps compute and the
    SBUF footprint stays ~176 KB/partition.  GEMM2 accumulates the full
    I contraction in PSUM (4 banks live) and a single outbound DMA
    writes outT.
    """
    import concourse.bacc as bacc
    import concourse.mybir as mybir
    import concourse.tile as tile

    dt = mybir.dt
    assert mm_dtype in ("bf16", "fp8")
    fp8 = mm_dtype == "fp8"
    # fp8: operands in float8e4 (TRN e4m3, max +-240), matmuls in DoubleRow
    # perf mode (2 k-chunks per instruction, ~1.4x PE throughput). PSUM
    # accumulation stays fp32; host pre-scales wi by 16 and wo by 64 to
    # center the fp8 dynamic range, and the host combine divides the gate
    # weights by 1024 to compensate (relu commutes with positive scale).
    DT = dt.float8e4 if fp8 else dt.bfloat16
    KSTEP = 2 if fp8 else 1
    PERF = mybir.MatmulPerfMode.DoubleRow if fp8 else None

    # Bacc (not raw Bass): its compile() pipeline splits multi-semaphore
    # waits into event-semaphore chains (TRN2 allows 1 wait/instruction)
    # and moves matmul waits onto ldweights.
    nc = bacc.Bacc("TRN2")
    xT = nc.declare_dram_parameter("xT", [H, CAP], DT, isOutput=False)
    wi = nc.declare_dram_parameter("wi", [H, I], DT, isOutput=False)
    wo = nc.declare_dram_parameter("wo", [I, H], DT, isOutput=False)
    outT = nc.declare_dram_parameter("outT", [H, CAP], dt.float32, isOutput=True)
    NPIECE = 4
    IPP = IC // NPIECE        # 8 i-chunks per wi piece

    with tile.TileContext(nc) as tc:
        from contextlib import ExitStack

        with ExitStack() as ctx:
            xpool = ctx.enter_context(tc.tile_pool(name="x", bufs=1))
            wpool = ctx.enter_context(tc.tile_pool(name="w", bufs=NPIECE))
            hpool = ctx.enter_context(tc.tile_pool(name="h", bufs=1))
            opool = ctx.enter_context(tc.tile_pool(name="o", bufs=1))
            # one shared 8-bank PSUM ring: GEMM1 draws 4 banks per i-chunk
            # pair, GEMM2 draws 4 per group — consecutive groups land on
            # different bank halves, so the DVE copies of group g overlap
            # the PE matmuls of group g+1 instead of serializing on the
            # same banks. ps2bank: tiles span 2 banks (both n-halves), so
            # each relu/copy drains a full [128,1024] row in one DVE op.
            pspool = ctx.enter_context(
                tc.tile_pool(name="ps", bufs=(4 if ps2bank else 8),
                             space="PSUM"))

            # x split into the two 512-token halves: GEMM1's first chain
            # only waits on the first 1MB DMA, not the full 2MB
            # (x_one=True: single tile, A/B experiment for SBUF access
            # patterns — rhs then comes from one tile for all matmuls)
            if x_one:
                x_whole = xpool.tile([P, KO, CAP], DT, name="x")
                x_sbs = [x_whole[:, :, n * NF:(n + 1) * NF] for n in range(NN)]
            else:
                x_sbs = [xpool.tile([P, KO, NF], DT, name=f"x{n}")
                         for n in range(NN)]
            # h split into two half-tiles: GEMM2's k=0..15 matmuls wait only
            # on the first half's relus (done ~70us before GEMM1 ends), so
            # the GEMM1->GEMM2 transition has no whole-tile barrier
            ICH = IC // 2
            h_sbs = [hpool.tile([P, ICH, CAP], DT, name=f"h{j}")
                     for j in range(2)]
            out_sb = opool.tile([P, KO, CAP], dt.float32)

            wi_r = wi.rearrange("(ko p) i -> p ko i", p=P)     # [128, 8, 4096]
            wo_r = wo.rearrange("(ki p) h -> p ki h", p=P)     # [128, 32, 1024]

          # fmt: off
          # noqa
            # wi piece sizes in i-chunks: a tiny first piece so the PE can
            # start GEMM1 after a ~0.5MB DMA instead of 4MB (the tile dep
            # is whole-tile); the ring still has NPIECE slots, with wi4
            # reusing wi0's slot after its chunks are consumed. All counts
            # even so GEMM1 can pair i-chunks for 4-bank interleaving.
            WI_SPLIT = [2, IC // NPIECE - 2] + [IC // NPIECE] * (NPIECE - 1)
            WI_START = [sum(WI_SPLIT[:j]) for j in range(len(WI_SPLIT))]

            xT_r = xT.rearrange("(ko p) n -> p ko n", p=P)

            # ablate: "full" | "empty" (loop overhead only) | "nog2"
            # (GEMM1+relu only) | "nog2pre" (GEMM1 only, wi preloaded
            # OUTSIDE the loop — separates DMA interference from relu
            # interference) | "notail" (skip PSUM->SBUF copies + out
            # DMA) | "nodma" (skip input DMAs; compute on stale SBUF) —
            # timing-attribution experiments, not for production use.
            do_dma = ablate != "nodma"
            do_g2 = ablate not in ("nog2", "nog2pre")
            do_tail = ablate not in ("nog2", "nog2pre", "notail")
            preload = ablate == "nog2pre"

            wi_pre = None
            if preload:
                # wi fully SBUF-resident (64KB/partition; fits since wo and
                # the w-ring go unused in this variant)
                wprepool = ctx.enter_context(
                    tc.tile_pool(name="wpre", bufs=1))
                wi_pre = []
                for p_, (i0, cnt) in enumerate(zip(WI_START, WI_SPLIT)):
                    wt = wprepool.tile([P, KO, cnt * P], DT,
                                       tag=f"wpre{p_}", name=f"wipre{p_}")
                    nc.sync.dma_start(
                        wt[:], wi_r[:, :, i0 * P:(i0 + cnt) * P])
                    wi_pre.append(wt)

            def _emit_body():
              if ablate == "empty":
                  nc.any.memset(out_sb[:, 0, 0:NF], 0.0)
                  return
              if do_dma:
                  if x_one:
                      nc.sync.dma_start(x_whole[:], xT_r[:])
                  else:
                      for n in range(NN):
                          nc.sync.dma_start(
                              x_sbs[n][:], xT_r[:, :, n * NF:(n + 1) * NF])
              if preload:
                  wi_pieces = wi_pre
              else:
                  wi_pieces = []
                  for p_, (i0, cnt) in enumerate(zip(WI_START, WI_SPLIT)):
                      wt = wpool.tile([P, KO, cnt * P], DT, tag="w",
                                      name=f"wi{p_}")
                      if do_dma:
                          nc.sync.dma_start(
                              wt[:], wi_r[:, :, i0 * P:(i0 + cnt) * P])
                      wi_pieces.append(wt)
              # ---- GEMM1: hT = relu(Wi.T @ X.T) ----
              wo_pieces = []
              HPP = H // NPIECE                                  # 256 H cols
              for p_, (i0, cnt) in enumerate(zip(WI_START, WI_SPLIT)):
                  wt = wi_pieces[p_]
                  for ir in range(0, cnt, 2):
                      # 4-way PSUM interleave over an i-chunk pair x the two
                      # n-tiles: consecutive matmuls hit 4 different banks
                      # (GEMM2's 4-deep pattern measures 238ns/matmul vs
                      # 305ns for the 2-deep version of this loop).
                      pair = [i0 + ir, i0 + ir + 1]
                      if ps2bank:
                          p2 = [pspool.tile([P, NN, NF], dt.float32,
                                            tag="ps", name=f"ps1_{i}")
                                for i in pair]
                          pts1 = [[p2[di][:, n, :] for n in range(NN)]
                                  for di in range(2)]
                      else:
                          pts1 = [
                              [
                                  pspool.tile([P, NF], dt.float32, tag="ps",
                                              name=f"ps1_{i}_{n}")
                                  for n in range(NN)
                              ]
                              for i in pair
                          ]
                      for k in range(0, KO, KSTEP):
                          for di in range(2):
                              if wide_mm:
                                  # one 1024-column matmul into the 2-bank
                                  # tile (requires x_one: rhs spans both
                                  # n-halves)
                                  nc.tensor.matmul(
                                      p2[di][:],
                                      wt[:, k:k + KSTEP,
                                         (ir + di) * P:(ir + di + 1) * P],
                                      x_whole[:, k:k + KSTEP, :],
                                      start=(k == 0),
                                      stop=(k == KO - KSTEP),
                                      perf_mode=PERF,
                                  )
                                  continue
                              for n in range(NN):
                                  nc.tensor.matmul(
                                      pts1[di][n][:],
                                      wt[:, k:k + KSTEP,
                                         (ir + di) * P:(ir + di + 1) * P],
                                      x_sbs[n][:, k:k + KSTEP, :],
                                      start=(k == 0),
                                      stop=(k == KO - KSTEP),
                                      perf_mode=PERF,
                                  )
                      for di, i in enumerate(pair):
                          hdst = h_sbs[i // ICH]
                          if ps2bank:
                              nc.vector.tensor_scalar_max(
                                  hdst[:, i % ICH, :], p2[di][:], 0.0)
                          else:
                              for n in range(NN):
                                  nc.vector.tensor_scalar_max(
                                      hdst[:, i % ICH, n * NF:(n + 1) * NF],
                                      pts1[di][n][:], 0.0
                                  )
                  # piece p_ fully consumed -> rotate the next wo piece into
                  # the freed ring slot (5 wi pieces, 4 wo pieces: skip the
                  # tiny piece 0)
                  if p_ >= 1 and do_g2:
                      g = p_ - 1
                      wot = wpool.tile([P, IC, HPP], DT, tag="w",
                                       name=f"wo{g}")
                      if do_dma:
                          nc.sync.dma_start(
                              wot[:], wo_r[:, :, g * HPP:(g + 1) * HPP])
                      wo_pieces.append(wot)

              if not do_g2:
                  return
              # ---- GEMM2: outT = Wo.T @ hT ----
              HGM = HPP // P                                     # 2 m per piece
              for g in range(NPIECE):
                  wt = wo_pieces[g]
                  if ps2bank:
                      q2 = [pspool.tile([P, NN, NF], dt.float32, tag="ps",
                                        name=f"ps2_{g}_{m}")
                            for m in range(HGM)]
                      pts = [[q2[m][:, n, :] for n in range(NN)]
                             for m in range(HGM)]
                  else:
                      pts = [
                          [
                              pspool.tile([P, NF], dt.float32, tag="ps",
                                          name=f"ps2_{g}_{m}_{n}")
                              for n in range(NN)
                          ]
                          for m in range(HGM)
                      ]
                  for k in range(0, IC, KSTEP):
                      for m in range(HGM):
                          if wide_mm:
                              nc.tensor.matmul(
                                  q2[m][:],
                                  wt[:, k:k + KSTEP, m * P:(m + 1) * P],
                                  h_sbs[k // ICH][:, k % ICH:k % ICH + KSTEP, :],
                                  start=(k == 0),
                                  stop=(k == IC - KSTEP),
                                  perf_mode=PERF,
                              )
                              continue
                          for n in range(NN):
                              nc.tensor.matmul(
                                  pts[m][n][:],
                                  wt[:, k:k + KSTEP, m * P:(m + 1) * P],
                                  h_sbs[k // ICH][
                                      :, k % ICH:k % ICH + KSTEP,
                                      n * NF:(n + 1) * NF],
                                  start=(k == 0),
                                  stop=(k == IC - KSTEP),
                                  perf_mode=PERF,
                              )
                  if not do_tail:
                      continue
                  # per-m copy + outbound DMA: each 0.5MB row leaves as soon
                  # as its copy lands, so only the last row's copy+DMA is a
                  # serial tail (~1.8us instead of ~3.5)
                  for m in range(HGM):
                      if ps2bank:
                          nc.vector.tensor_copy(
                              out_sb[:, g * HGM + m, :], q2[m][:])
                      else:
                          for n in range(NN):
                              nc.vector.tensor_copy(
                                  out_sb[:, g * HGM + m, n * NF:(n + 1) * NF],
                                  pts[m][n][:])
                      nc.sync.dma_start(
                          outT.rearrange("(ko p) n -> p ko n", p=P)[
                              :, g * HGM + m:g * HGM + m + 1, :],
                          out_sb[:, g * HGM + m:g * HGM + m + 1, :])

            if loop_repeat is not None:
                # device-side repeat loop: used only for timing (the slope
                # d(wall)/d(R) isolates per-iteration device time from the
                # ~70ms axon dispatch round-trip). hint_engines arms the
                # back-edge branch prefetch (the >256-instruction body
                # otherwise stalls ~4us on the IRAM fetch).
                with tc.For_i(0, loop_repeat, 1,
                              hint_engines=(mybir.EngineType.PE,
                                            mybir.EngineType.DVE),
                              staggered_reset=staggered):
                    _emit_body()
            else:
                for _rep in range(repeat):
                    _emit_body()
    nc.compile()
    return nc


def _get_nc(mm_dtype: str):
    if mm_dtype not in _CACHE:
        _CACHE[mm_dtype] = _build_nc(mm_dtype)
    return _CACHE[mm_dtype]


def _routing(x_flat: np.ndarray, router_w: np.ndarray, router_b: np.ndarray):
    """Replicate the reference router bit-for-bit where possible (jax CPU),
    returning top-2 values/indices [N_TOK, 2] (fp32/int)."""
    try:
        import jax
        import jax.numpy as jnp

        cpu = jax.devices("cpu")[0]
        with jax.default_device(cpu):
            xj = jnp.asarray(x_flat.reshape(B, S, H))
            logits = jnp.einsum("bsh,eh->bse", xj, jnp.asarray(router_w)) \
                + jnp.asarray(router_b)
            wflat = jax.nn.softmax(logits, axis=-1).reshape(N_TOK, E)
            topv, topi = jax.lax.top_k(wflat, TOP_K)
            return np.asarray(topv), np.asarray(topi)
    except Exception:
        # numpy fallback (float64 logits for a stable ordering)
        logits = x_flat.astype(np.float64) @ router_w.astype(np.float64).T \
            + router_b.astype(np.float64)
        m = logits.max(axis=1, keepdims=True)
        ex = np.exp(logits - m)
        wflat = (ex / ex.sum(axis=1, keepdims=True)).astype(np.float32)
        topi = np.argsort(-wflat, axis=1, kind="stable")[:, :TOP_K]
        topv = np.take_along_axis(wflat, topi, axis=1)
        return topv, topi


def _dispatch(x_flat, topv, topi):
    """Capacity-limited dispatch (exact reference order semantics).

    Returns (pos, disp_T): pos[t, e] = rank of t among selectors of e
    (token order); disp_T[e] = x of the first CAP selectors, transposed
    to [H, CAP]."""
    mask = np.zeros((N_TOK, E), dtype=bool)
    rows = np.arange(N_TOK)
    mask[rows[:, None], topi] = True
    expert_mask = mask[:, :ER]                       # [N, 7]
    pos = np.cumsum(expert_mask, axis=0, dtype=np.int32) - 1

    disp_T = np.zeros((ER, H, CAP), dtype=np.float32)
    for e in range(ER):
        idx_e = np.nonzero(expert_mask[:, e])[0][:CAP]
        disp_T[e, :, :len(idx_e)] = x_flat[idx_e].T
    return pos, disp_T


def _make_in_maps(disp_T, experts_inter, experts_out, mm_dtype=None):
    """Per-core device input maps + the output scale to undo fp8 pre-scaling."""
    import ml_dtypes

    mm_dtype = mm_dtype or MM_DTYPE
    if mm_dtype == "fp8":
        f8 = ml_dtypes.float8_e4m3
        cast_x = lambda a: np.ascontiguousarray(a.astype(f8))
        cast_wi = lambda a: np.ascontiguousarray((a * WI_SCALE).astype(f8))
        cast_wo = lambda a: np.ascontiguousarray((a * WO_SCALE).astype(f8))
        out_scale = 1.0 / (WI_SCALE * WO_SCALE)
    else:
        bf = lambda a: np.ascontiguousarray(a.astype(ml_dtypes.bfloat16))
        cast_x = cast_wi = cast_wo = bf
        out_scale = 1.0

    in_maps = []
    for c in range(N_CORES):
        e = c if c < ER else 0
        in_maps.append({
            "xT": cast_x(disp_T[e]),
            "wi": cast_wi(experts_inter[e]),
            "wo": cast_wo(experts_out[e]),
        })
    return in_maps, out_scale


def kernel(x, router_w, router_b, experts_inter, experts_out):
    from concourse.bass_utils import run_bass_kernel_spmd

    x = np.ascontiguousarray(np.asarray(x, dtype=np.float32))
    router_w = np.asarray(router_w, dtype=np.float32)
    router_b = np.asarray(router_b, dtype=np.float32)
    experts_inter = np.asarray(experts_inter, dtype=np.float32)
    experts_out = np.asarray(experts_out, dtype=np.float32)

    x_flat = x.reshape(N_TOK, H)
    topv, topi = _routing(x_flat, router_w, router_b)
    pos, disp_T = _dispatch(x_flat, topv, topi)
    rows = np.arange(N_TOK)

    mm_dtype = MM_DTYPE
    in_maps, out_scale = _make_in_maps(disp_T, experts_inter, experts_out,
                                       mm_dtype)

    nc = _get_nc(mm_dtype)
    trace = bool(int(os.environ.get("MOE_TRACE", "0")))
    res = run_bass_kernel_spmd(nc, in_maps, list(range(N_CORES)), trace=trace)
    global LAST_RESULT
    LAST_RESULT = res
    out_T = np.stack([res.results[e]["outT"] for e in range(ER)])  # [7,H,cap]

    # ---- host combine: pure gathers ----
    out_flat = np.ascontiguousarray(out_T.transpose(0, 2, 1)).reshape(
        ER * CAP, H)
    out_ext = np.vstack([out_flat, np.zeros((1, H), dtype=np.float32)])

    combined = np.zeros_like(x_flat)
    noop_w = np.zeros(N_TOK, dtype=np.float32)
    for k in range(TOP_K):
        e_k = topi[:, k]
        v_k = topv[:, k]
        is_noop = e_k == ER
        noop_w += np.where(is_noop, v_k, 0.0).astype(np.float32)
        p_k = pos[rows, np.minimum(e_k, ER - 1)]
        ok = (~is_noop) & (p_k < CAP)
        slot = np.where(ok, np.minimum(e_k, ER - 1) * CAP + p_k, ER * CAP)
        combined += out_ext[slot] * (np.where(ok, v_k, 0.0) * out_scale)[:, None]
    combined += x_flat * noop_w[:, None]

    return combined.reshape(B, S, H)

